# revision 32
# baseline (speedup 1.0000x reference)
"""Trainium2 Bass kernel for nn_Attention_Layer_78855599554595.

GQA attention layer: QKV proj -> causal GQA attention (16 heads, 4 kv heads,
E=128) -> out proj -> exact GELU -> residual -> LayerNorm.  B=2, L=2048, D=2048.

Sharding: zero-communication interleaved sequence parallelism.
  - 8 cores = 2 batches x 4 cores/batch.
  - Core j of a batch owns query rows in g=64-row blocks strided by 4:
    global blocks {j, j+4, ..., j+28} (512 rows).  SPMD: one program,
    per-core data; causal structure is identical across cores.
  - Each core computes K/V for its full batch (redundant 4x, but cheaper
    than any on-chip collective here).
  - Scores are computed transposed (S^T: keys on partitions, queries on
    the free axis) so softmax needs no transposes; no max-subtraction
    (scores are O(+-10); exp is fp32-safe).

Perf notes (cost-model driven):
  - fp32 and fp32r share a bit layout: all fp32 DRAM data is DMA'd once
    and bitcast to fp32r at matmul time (no convert copies).
  - wk/wv are pre-rounded to bf16 on the host and shipped as uint16 bit
    patterns (bf16 moving operands run at full PE rate at any tile size).
  - The causal mask is added on the PE itself (identity-stationary matmul
    with a bf16 mask as the moving operand) inside the score accumulation
    group, keeping DVE off the attention critical path.
  - Attention is software-pipelined: score+mask+exp for block i issue
    ahead of the pl/pctx consumption of block i-2, so the in-order PE
    queue never waits on the ACT exp.
  - LayerNorm stats use DVE bn_stats/bn_aggr (one pass, no ACT square).
  - All constants (ones, identities, eps) come from one host tensor: the
    Pool engine issues only SWDGE DMAs, and no engine idles on memsets.
"""

import sys

sys.path.insert(0, "/opt/trn_rl_repo")

import numpy as np

from contextlib import ExitStack
from dataclasses import dataclass, field

from concourse import bacc, mybir, tile

F32 = mybir.dt.float32
R = mybir.dt.float32r
BF = mybir.dt.bfloat16
U16 = mybir.dt.uint16
NEG = -1.0e9
AF = mybir.ActivationFunctionType


@dataclass(frozen=True)
class Cfg:
    L: int = 2048          # sequence length (per batch)
    D: int = 2048          # model dim
    H: int = 16            # query heads
    KV: int = 4            # kv heads
    E: int = 128           # head dim (= partition width)
    mm_dt: object = field(default=mybir.dt.float32r)
    act: object = field(default=None)  # None -> exact GELU
    trivial_affine: bool = False  # gamma==1, beta==0, bo==0: skip those ops

    @property
    def g(self):           # q block granularity (32 blocks across L)
        return self.L // 32

    @property
    def KB(self):          # key block size = 4*g
        return self.L // 8

    @property
    def KSS(self):         # key subtile (partition) size
        return min(self.KB, 128)

    @property
    def ST(self):          # key subtiles per key block
        return max(1, self.KB // 128)

    @property
    def QR(self):          # query rows per core
        return self.L // 4

    @property
    def KT(self):          # contraction tiles over D
        return self.D // 128

    @property
    def RT(self):          # 128-row tiles of the core's q rows
        return self.QR // 128

    @property
    def OC(self):          # out-proj / LN column chunk
        return min(self.D, 512)


def build_program(cfg: Cfg):
    """Build the single-core SPMD Bass program. Returns finalized nc."""
    L, D, H, KV, E = cfg.L, cfg.D, cfg.H, cfg.KV, cfg.E
    g, KB, KSS, ST, QR, KT, RT = (cfg.g, cfg.KB, cfg.KSS, cfg.ST, cfg.QR,
                                  cfg.KT, cfg.RT)
    OC = cfg.OC
    NOC = D // OC
    KVE = KV * E
    act_fn = cfg.act if cfg.act is not None else AF.Gelu
    inv_sqrt_e = 1.0 / float(np.sqrt(E))

    nc = bacc.Bacc(None, target_bir_lowering=False)

    # ---- DRAM I/O (per-core data; same names on every core) ----
    xtu = nc.dram_tensor("xtu", [D, L], U16, kind="ExternalInput")    # x[b].T bf16
    xtqu = nc.dram_tensor("xtqu", [D, QR], U16, kind="ExternalInput")  # bf16
    xq = nc.dram_tensor("xq", [QR, D], F32, kind="ExternalInput")     # rows at q rows
    wqu = nc.dram_tensor("wqu", [D, H * E], U16, kind="ExternalInput")  # bf16
    wku = nc.dram_tensor("wku", [D, KVE], U16, kind="ExternalInput")  # bf16 bits
    wvu = nc.dram_tensor("wvu", [D, KVE], U16, kind="ExternalInput")  # bf16 bits
    wou = nc.dram_tensor("wou", [H * E, D], U16, kind="ExternalInput")  # bf16
    bo2 = nc.dram_tensor("bo2", [2, D], F32, kind="ExternalInput")  # bo row + zero row
    gmb = nc.dram_tensor("gmb", [128, D], F32, kind="ExternalInput")  # gamma bcast
    btb = nc.dram_tensor("btb", [128, D], F32, kind="ExternalInput")  # beta bcast
    # combined f32 consts: [0:130] ones, [130:258] unused, [258] eps,
    # [384:400] bqT, [400:404] bkT, [404:916] bvb  (one DMA)
    cstA = nc.dram_tensor("cstA", [128, 916], F32, kind="ExternalInput")
    # combined bf16-bit consts: [0:128] identity, [128:256] maskd (S^T,
    # st-major), [256:2304] maskp ((kb,st)-major)  (one DMA)
    cstB = nc.dram_tensor("cstB", [128, 2304], U16, kind="ExternalInput")
    out = nc.dram_tensor("out", [QR, D], F32, kind="ExternalOutput")

    with tile.TileContext(nc) as tc, ExitStack() as top:
        # ---- persistent pools (stack order matters for SBUF reuse) ----
        const = top.enter_context(tc.tile_pool(name="const", bufs=1))
        qt_stack = top.enter_context(ExitStack())
        qt_pool = qt_stack.enter_context(tc.tile_pool(name="qtp", bufs=1))
        kvq_pool = top.enter_context(tc.tile_pool(name="kvq", bufs=1))
        xtq_stack = ExitStack()
        xtq_pool = xtq_stack.enter_context(tc.tile_pool(name="xtqp", bufs=1))
        wq_stack = ExitStack()
        wq_pool = wq_stack.enter_context(
            tc.tile_pool(name="wqstage", bufs=3))

        # constants (two DMAs from host; no memsets anywhere)
        cstf_t = const.tile([128, 916], F32)
        cstb_t = const.tile([128, 2304], U16)
        warm = const.tile([1, 2], F32)
        nc.gpsimd.dma_start(out=cstf_t[:], in_=cstA[:])
        nc.gpsimd.dma_start(out=cstb_t[:], in_=cstB[:])
        bq_t = cstf_t[:, 384:400]
        identb = cstb_t[:, 0:128]
        # fp32r matmul operands must be produced by a rounding instruction
        # (BIR verifier rule), so the ones constants get one tiny convert.
        ones_r = const.tile([128, 130], R)
        nc.vector.tensor_copy(ones_r[:], cstf_t[:, 0:130])
        ones2 = ones_r[:, 0:2]          # [128, 2] ones (pl lhsT)
        ones1 = ones_r[:1, 2:130]       # [1, 128] ones (broadcast lhsT)
        eps_c = cstf_t[:, 258:259]      # [128, 1] eps
        # Prime the Exp activation-table set before any other ACT op so the
        # one loaded set covers Copy/Identity (phases 1-2) and Exp (phase 3).
        nc.scalar.activation(warm[:], cstf_t[:1, 0:2], AF.Exp)

        # persistent activations: K^T, V (natural) per kv head; Q^T per head
        kT = [kvq_pool.tile([E, L], R, tag=f"kT{kv}", name=f"kT{kv}")
              for kv in range(KV)]
        vN = [kvq_pool.tile([KSS, L // KSS, E], R, tag=f"vN{kv}",
                            name=f"vN{kv}") for kv in range(KV)]

        # x^T at q rows, prefetched during phase 1 (bf16 bits)
        xtq_t = xtq_pool.tile([128, KT, QR], U16)

        # ================= Phase 1: K/V projections (full batch rows) ======
        with ExitStack() as ph:
            wkv_pool = ph.enter_context(tc.tile_pool(name="wkv", bufs=1))
            stage = ph.enter_context(tc.tile_pool(name="stage1", bufs=3))
            cst1 = ph.enter_context(tc.tile_pool(name="cst1", bufs=1))
            ps1 = ph.enter_context(tc.tile_pool(name="ps1", bufs=2, space="PSUM"))

            wk_t = wkv_pool.tile([128, KT, KVE], U16, name="wk_t")
            wv_t = wkv_pool.tile([128, KT, KVE], U16, name="wv_t")

            def load_wkv(c):
                nc.sync.dma_start(
                    out=wk_t[:, 4 * c:4 * (c + 1), :],
                    in_=wku[c * 512:(c + 1) * 512, :]
                    .rearrange("(k p) c -> p k c", p=128))
                nc.sync.dma_start(
                    out=wv_t[:, 4 * c:4 * (c + 1), :],
                    in_=wvu[c * 512:(c + 1) * 512, :]
                    .rearrange("(k p) c -> p k c", p=128))

            load_wkv(0)
            bkT_t = cstf_t[:, 400:404]
            bvb_t = cstf_t[:, 404:916]

            for rt in range(L // 128):
                xs = stage.tile([128, KT, 128], U16, tag="xs")
                nc.sync.dma_start(
                    out=xs[:],
                    in_=xtu[:, rt * 128:(rt + 1) * 128]
                    .rearrange("(k p) r -> p k r", p=128))
                # Remaining weight chunks are emitted right after xs(0) (so
                # every consumer follows its producer in program order), but
                # land in the DMA pipe after it; xtq prefetch follows later.
                if rt == 0:
                    for c in (1, 2, 3):
                        load_wkv(c)
                elif 4 <= rt <= 7:
                    c = rt - 4
                    nc.sync.dma_start(
                        out=xtq_t[:, 4 * c:4 * (c + 1), :],
                        in_=xtqu[c * 512:(c + 1) * 512, :]
                        .rearrange("(k p) r -> p k r", p=128))
                # K^T computed directly (wk stationary, x^T moving): no
                # PE transposes, and the eviction carries the bias.  Each kv
                # head accumulates in its own PSUM bank: a matmul's start
                # flag pending-zeroes the whole 2KB bank, so independent
                # accumulations must not share one.
                pV = ps1.tile([128, KVE], F32, tag="pV")
                pKTs = [ps1.tile([E, 128], F32, tag=f"pKT{kv}", bufs=1,
                                 name=f"pKT{kv}") for kv in range(KV)]
                for kt in range(KT):
                    nc.tensor.matmul(pV[:], xs[:, kt, :].bitcast(BF),
                                     wv_t[:, kt, :].bitcast(BF),
                                     start=(kt == 0), stop=(kt == KT - 1))
                for kt in range(KT):
                    for kv in range(KV):
                        nc.tensor.matmul(
                            pKTs[kv][:],
                            wk_t[:, kt, kv * E:(kv + 1) * E].bitcast(BF),
                            xs[:, kt, :].bitcast(BF),
                            start=(kt == 0), stop=(kt == KT - 1),
                            skip_group_check=True)
                # V natural: evict (+bias) straight into vN, rounding to fp32r
                for kv in range(KV):
                    nc.vector.tensor_add(
                        vN[kv][:, rt, :], pV[:, kv * E:(kv + 1) * E],
                        bvb_t[:, kv * E:(kv + 1) * E])
                for kv in range(KV):
                    nc.scalar.activation(
                        kT[kv][:, rt * 128:(rt + 1) * 128], pKTs[kv][:],
                        AF.Identity, bias=bkT_t[:, kv:kv + 1])

        # ================= Phase 2: Q^T projection (core's rows) ===========
        qT = [qt_pool.tile([E, QR], R, tag=f"qT{h}", name=f"qT{h}")
              for h in range(H)]
        with ExitStack() as ph:
            ps2 = ph.enter_context(tc.tile_pool(name="ps2", bufs=1, space="PSUM"))
            HB = 4
            for hb in range(H // HB):
                pqs = [ps2.tile([E, QR], F32, tag=f"pq{hh}", name=f"pq{hh}")
                       for hh in range(HB)]
                for c in range(KT // 4):
                    wqs = wq_pool.tile([128, 4, HB * E], U16, tag="wqs")
                    nc.sync.dma_start(
                        out=wqs[:],
                        in_=wqu[c * 512:(c + 1) * 512,
                                hb * HB * E:(hb + 1) * HB * E]
                        .rearrange("(k p) c -> p k c", p=128))
                    for k4 in range(4):
                        kt = 4 * c + k4
                        for hh in range(HB):
                            nc.tensor.matmul(
                                pqs[hh][:],
                                wqs[:, k4, hh * E:(hh + 1) * E].bitcast(BF),
                                xtq_t[:, kt, :].bitcast(BF),
                                start=(kt == 0), stop=(kt == KT - 1))
                for hh in range(HB):
                    # split the evictions across ACT and DVE so the last
                    # group's eviction tail is short
                    h = hb * HB + hh
                    if hh % 2:
                        nc.scalar.activation(
                            qT[h][:], pqs[hh][:], AF.Identity,
                            bias=bq_t[:, h:h + 1])
                    else:
                        nc.vector.tensor_scalar_add(
                            qT[h][:], pqs[hh][:], bq_t[:, h:h + 1])
        wq_stack.close()
        xtq_stack.close()
        # wo prefetch pool: reuses the just-released xtq/wq SBUF region, so
        # its (Pool-queue) DMAs start right after phase 2 and run through
        # phase 3.
        wo_stack = top.enter_context(ExitStack())
        wo_pool = wo_stack.enter_context(tc.tile_pool(name="wop", bufs=2))

        # ================= Phase 3: attention ==============================
        # Flat software pipeline over (head, key-block) steps: the score +
        # mask + exp of step s issue ahead of the pl/pctx consumption of
        # step s-2, and each head's normalize runs inside the next head's
        # stream, so the in-order PE queue never waits on ACT.
        ctxT = [None] * H
        with ExitStack() as ph:
            ps_ctx = ph.enter_context(
                tc.tile_pool(name="psctx", bufs=2, space="PSUM"))
            ps_m = ph.enter_context(tc.tile_pool(name="psm", bufs=2, space="PSUM"))
            ps_s = ph.enter_context(tc.tile_pool(name="pss", bufs=2, space="PSUM"))
            exp_pool = ph.enter_context(tc.tile_pool(name="expp", bufs=4))
            lso_pool = ph.enter_context(tc.tile_pool(name="lso", bufs=2))

            q0s = [min(g * kb, QR // 2) for kb in range(8)]
            qcs = [QR - q0 for q0 in q0s]
            LAG = 2
            steps = [(h, kb) for h in range(H) for kb in range(8)]
            state = {}  # h -> (pl, pctx, ess)

            def produce(h, kb):
                kv = h % KV
                q0, qc = q0s[kb], qcs[kb]
                k0 = kb * KB
                pS = ps_s.tile([KSS, ST, QR], F32, tag="pS")
                for st in range(ST):
                    nc.tensor.matmul(
                        pS[:, st, :qc],
                        kT[kv][:, k0 + st * KSS:k0 + (st + 1) * KSS],
                        qT[h][:, q0:], start=True, stop=False,
                        skip_group_check=True)
                # causal mask folded into the accumulation group on PE:
                # pS[:, st, :w] += I^T @ mask  (bf16 moving, full rate)
                for st in range(ST):
                    if kb < 4:
                        w = g
                        m0 = 128 + st * g
                    else:
                        w = g * (kb - 3)
                        m0 = 256 + ((kb - 4) * ST + st) * (QR // 2)
                    nc.tensor.matmul(
                        pS[:, st, :w], identb.bitcast(BF),
                        cstb_t[:, m0:m0 + w].bitcast(BF), start=False,
                        stop=True, skip_group_check=True)
                eS = exp_pool.tile([KSS, ST, QR], R, tag="eS", bufs=4)
                nc.scalar.activation(eS[:, :, :qc], pS[:, :, :qc], AF.Exp,
                                     scale=inv_sqrt_e)
                state[h][2].append(eS)

            def consume(h, j):
                kv = h % KV
                pl, pctx, ess = state[h]
                q0, qc = q0s[j], qcs[j]
                for st in range(ST):
                    first = (j == 0 and st == 0)
                    lst = (j == 7 and st == ST - 1)
                    nc.tensor.matmul(
                        pl[:, q0:], ones2, ess[j][:, st, :qc],
                        start=first, stop=lst, skip_group_check=True)
                    nc.tensor.matmul(
                        pctx[:, q0:], vN[kv][:, 2 * j + st, :],
                        ess[j][:, st, :qc],
                        start=first, stop=lst, skip_group_check=True)

            def epilogue(h):
                # normalize: cT = pctx * broadcast(1/l)
                pl, pctx, _ = state.pop(h)
                rl = lso_pool.tile([2, QR], R, tag="rl")
                rlf = lso_pool.tile([1, QR], F32, tag="rlf")
                nc.vector.reciprocal_approx_fast(rlf[:], pl[:1, :])
                nc.vector.tensor_copy(rl[:1, :], rlf[:])
                prb = ps_m.tile([E, QR], F32, tag="m")
                nc.tensor.matmul(prb[:], ones1, rl[:1, :],
                                 start=True, stop=True)
                rb_s = lso_pool.tile([E, QR], F32, tag="rbs")
                nc.scalar.activation(rb_s[:], prb[:], AF.Copy)
                cT = qt_pool.tile([E, QR], BF, tag=f"qT{h}", name=f"cT{h}")
                nc.vector.tensor_mul(cT[:], pctx[:], rb_s[:])
                ctxT[h] = cT

            for s in range(len(steps) + LAG):
                if s < len(steps):
                    h, kb = steps[s]
                    if kb == 0:
                        state[h] = (
                            ps_m.tile([2, QR], F32, tag="m", name=f"pl{h}"),
                            ps_ctx.tile([E, QR], F32, tag="pctx",
                                        name=f"pctx{h}"),
                            [])
                    produce(h, kb)
                if s >= LAG:
                    h, j = steps[s - LAG]
                    consume(h, j)
                    if j == 7:
                        epilogue(h)

        # ============ Phase 4: out-proj + GELU + residual + LayerNorm ======
        r_stack = top.enter_context(ExitStack())
        rfull_pool = r_stack.enter_context(tc.tile_pool(name="rfull", bufs=1))
        stat4 = r_stack.enter_context(tc.tile_pool(name="stat4", bufs=1))
        r_full = [rfull_pool.tile([128, D], F32, tag=f"rf{rt}", name=f"rf{rt}")
                  for rt in range(RT)]
        stat6 = [stat4.tile([128, NOC, 6], F32, tag=f"st{rt}", name=f"st{rt}")
                 for rt in range(RT)]
        with ExitStack() as ph:
            ps_y = ph.enter_context(tc.tile_pool(name="psy", bufs=2, space="PSUM"))
            ep_pool = ph.enter_context(tc.tile_pool(name="epp", bufs=3))
            cst4 = ph.enter_context(tc.tile_pool(name="cst4", bufs=1))
            ln_pool = ph.enter_context(tc.tile_pool(name="lnp", bufs=2))
            st_pool = ph.enter_context(tc.tile_pool(name="stp", bufs=2))
            gb_pool = ph.enter_context(tc.tile_pool(name="gbp", bufs=2))

            if not cfg.trivial_affine:
                bo2f = cst4.tile([2, D], F32)
                nc.sync.dma_start(out=bo2f[:], in_=bo2[:])
                bo2r = cst4.tile([2, D], R)
                nc.vector.tensor_copy(bo2r[:], bo2f[:])

            # LayerNorm epilogue.  rstd = rsqrt(var+eps) is computed per
            # row-tile on the DVE via Newton iterations seeded from 1/v
            # (converges for all v > 1/3; var+eps here is ~1.5), so no ACT
            # Sqrt is needed: the Gelu table set stays loaded, and each
            # row-tile normalizes + stores as soon as its own stats land.
            mv4 = st_pool.tile([128, RT, 2], F32, name="mv4")

            def ln_rt(rt):
                nc.vector.bn_aggr(mv4[:, rt, :], stat6[rt][:])
                vv = st_pool.tile([128, 1], F32, tag=f"vv{rt}", name=f"vv{rt}")
                nc.vector.tensor_scalar_add(vv[:], mv4[:, rt, 1:2], eps_c)
                y = st_pool.tile([128, 1], F32, tag=f"y{rt}", name=f"y{rt}")
                nc.vector.reciprocal(y[:], vv[:])
                t = st_pool.tile([128, 1], F32, tag=f"t{rt}", name=f"t{rt}")
                for _ in range(4):
                    nc.vector.tensor_mul(t[:], y[:], y[:])
                    nc.vector.tensor_mul(t[:], t[:], vv[:])
                    nc.vector.tensor_scalar(
                        t[:], t[:], -0.5, 1.5,
                        op0=mybir.AluOpType.mult, op1=mybir.AluOpType.add)
                    nc.vector.tensor_mul(y[:], y[:], t[:])
                nmr = st_pool.tile([128, 1], F32, tag=f"nm{rt}",
                                   name=f"nm{rt}")
                nc.vector.tensor_mul(nmr[:], mv4[:, rt, 0:1], y[:])
                nc.vector.tensor_scalar_mul(nmr[:], nmr[:], -1.0)
                for c in range(NOC):
                    sl = slice(c * OC, (c + 1) * OC)
                    rchunk = r_full[rt][:, sl]
                    if (c + rt) % 2:
                        nc.scalar.activation(
                            rchunk, rchunk, AF.Identity,
                            scale=y[:], bias=nmr[:])
                    else:
                        nc.vector.tensor_scalar(
                            rchunk, rchunk, y[:], nmr[:],
                            op0=mybir.AluOpType.mult, op1=mybir.AluOpType.add)
                    if not cfg.trivial_affine:
                        gm_c = gb_pool.tile([128, OC], F32, tag="gmc")
                        bt_c = gb_pool.tile([128, OC], F32, tag="btc")
                        nc.sync.dma_start(out=gm_c[:], in_=gmb[:, sl])
                        nc.sync.dma_start(out=bt_c[:], in_=btb[:, sl])
                        nc.vector.tensor_mul(rchunk, rchunk, gm_c[:])
                        nc.vector.tensor_add(rchunk, rchunk, bt_c[:])
                    nc.sync.dma_start(out=out[rt * 128:(rt + 1) * 128, sl],
                                      in_=rchunk)

            HW4 = 4  # h-chunk per wo load piece
            for oc in range(NOC):
                # load wo[:, oc] transposed-tiled: (128, H, OC) in pieces
                woc = wo_pool.tile([128, H, OC], U16, tag="woc")
                for pc in range(H // HW4):
                    nc.gpsimd.dma_start(
                        out=woc[:, pc * HW4:(pc + 1) * HW4, :],
                        in_=wou[pc * HW4 * E:(pc + 1) * HW4 * E,
                                oc * OC:(oc + 1) * OC]
                        .rearrange("(h p) c -> p h c", p=128))
                for rt in range(RT):
                    py = ps_y.tile([128, OC], F32, tag="py")
                    for h in range(H):
                        nc.tensor.matmul(
                            py[:], ctxT[h][:, rt * 128:(rt + 1) * 128],
                            woc[:, h, :].bitcast(BF), start=(h == 0),
                            stop=(cfg.trivial_affine and h == H - 1))
                    if not cfg.trivial_affine:
                        nc.tensor.matmul(
                            py[:], ones1,
                            bo2r[:1, oc * OC:(oc + 1) * OC],
                            start=False, stop=True, skip_group_check=True)
                    t2 = ep_pool.tile([128, OC], F32, tag="t2")
                    nc.scalar.activation(t2[:], py[:], act_fn)
                    xqt = ep_pool.tile([128, OC], F32, tag="xqt")
                    nc.sync.dma_start(
                        out=xqt[:],
                        in_=xq[rt * 128:(rt + 1) * 128, oc * OC:(oc + 1) * OC])
                    rchunk = r_full[rt][:, oc * OC:(oc + 1) * OC]
                    nc.vector.tensor_add(rchunk, t2[:], xqt[:])
                    nc.vector.bn_stats(stat6[rt][:, oc, :], rchunk)
                    if oc == NOC - 1:
                        ln_rt(rt)

    nc.finalize()
    return nc


# ---------------------------------------------------------------------------
# host-side mask construction + sharding
# ---------------------------------------------------------------------------

def _bf16_bits(a):
    u = np.ascontiguousarray(a, np.float32).view(np.uint32)
    return ((u + 0x8000) >> 16).astype(np.uint16)


def build_masks(cfg: Cfg, j: int):
    g, KB, QR, KSS, ST = cfg.g, cfg.KB, cfg.QR, cfg.KSS, cfg.ST
    c = np.arange(KB)[:, None]
    r = np.arange(g)[None, :]
    maskd = np.where(c <= j * g + r, 0.0, NEG).astype(np.float32)
    maskp = np.zeros((4, KB, QR // 2), np.float32)
    m = np.arange(QR // 2)
    i_of_m = 4 + m // g
    r_of_m = m % g
    for kbi, kb in enumerate(range(4, 8)):
        block = np.zeros((KB, QR // 2), np.float32)
        block[:, i_of_m < kb] = NEG
        dcols = np.where(i_of_m == kb)[0]
        block[:, dcols] = np.where(c <= j * g + r_of_m[dcols][None, :], 0.0, NEG)
        maskp[kbi] = block
    # rearrange to partitioned S^T layout and convert to bf16 bit patterns
    maskdu = _bf16_bits(maskd.reshape(ST, KSS, g).transpose(1, 0, 2))
    maskpu = _bf16_bits(
        maskp.reshape(4, ST, KSS, QR // 2).transpose(2, 0, 1, 3))
    return (np.ascontiguousarray(maskdu.reshape(KSS, ST * g)),
            np.ascontiguousarray(maskpu.reshape(KSS, 4 * ST * (QR // 2))))


def q_rows(cfg: Cfg, j: int):
    g = cfg.g
    return np.concatenate(
        [np.arange((j + 4 * i) * g, (j + 4 * i + 1) * g) for i in range(8)])


def make_in_map(cfg: Cfg, shared, x, b, j):
    rows = q_rows(cfg, j)
    xb = np.asarray(x, np.float32)[b]
    xbT = np.ascontiguousarray(xb.T)
    maskdu, maskpu = build_masks(cfg, j)
    cstB = np.empty((cfg.KSS, 2304), np.uint16)
    cstB[:, 0:128] = shared["_identu"]
    cstB[:, 128:256] = maskdu
    cstB[:, 256:2304] = maskpu
    d = dict(
        shared,
        xtu=_bf16_bits(xbT),
        xtqu=_bf16_bits(xbT[:, rows]),
        xq=np.ascontiguousarray(xb[rows]),
        cstB=cstB,
    )
    del d["_identu"]
    return d


def make_shared(cfg: Cfg, Wq, bq, Wk, bk, Wv, bv, Wo, bo, gamma, beta):
    H, KV, E, D = cfg.H, cfg.KV, cfg.E, cfg.D
    cstA = np.zeros((128, 916), np.float32)
    cstA[:, :130] = 1.0
    cstA[:, 258] = 1e-5
    cstA[:, 384:400] = np.asarray(bq, np.float32).reshape(H, E).T
    cstA[:, 400:404] = np.asarray(bk, np.float32).reshape(KV, E).T
    cstA[:, 404:916] = np.asarray(bv, np.float32)[None, :]
    return {
        "wqu": _bf16_bits(Wq),
        "wku": _bf16_bits(Wk),
        "wvu": _bf16_bits(Wv),
        "wou": _bf16_bits(Wo),
        "bo2": np.ascontiguousarray(
            np.stack([np.asarray(bo, np.float32),
                      np.zeros(D, np.float32)])),
        "gmb": np.ascontiguousarray(
            np.broadcast_to(np.asarray(gamma, np.float32), (128, D))),
        "btb": np.ascontiguousarray(
            np.broadcast_to(np.asarray(beta, np.float32), (128, D))),
        "cstA": cstA,
        "_identu": _bf16_bits(np.eye(128, dtype=np.float32)),
    }


def assemble(cfg: Cfg, results, B):
    out = np.empty((B, cfg.L, cfg.D), np.float32)
    for core in range(4 * B):
        b, j = divmod(core, 4)
        out[b, q_rows(cfg, j)] = results[core]["out"]
    return out


_NC_CACHE = {}


def kernel(x, Wq, bq, Wk, bk, Wv, bv, Wo, bo, gamma, beta):
    from concourse.bass_utils import run_bass_kernel_spmd

    trivial = bool(
        np.all(np.asarray(gamma) == 1.0) and np.all(np.asarray(beta) == 0.0)
        and np.all(np.asarray(bo) == 0.0))
    cfg = Cfg(trivial_affine=trivial)
    if cfg not in _NC_CACHE:
        _NC_CACHE[cfg] = build_program(cfg)
    nc = _NC_CACHE[cfg]
    shared = make_shared(cfg, Wq, bq, Wk, bk, Wv, bv, Wo, bo, gamma, beta)
    in_maps = [make_in_map(cfg, shared, x, *divmod(core, 4))
               for core in range(8)]
    res = run_bass_kernel_spmd(nc, in_maps, list(range(8)))
    return assemble(cfg, res.results, 2)


# revision 35
# speedup vs baseline: 1.0077x; 1.0077x over previous
"""Trainium2 Bass kernel for nn_Attention_Layer_78855599554595.

GQA attention layer: QKV proj -> causal GQA attention (16 heads, 4 kv heads,
E=128) -> out proj -> exact GELU -> residual -> LayerNorm.  B=2, L=2048, D=2048.

Sharding: zero-communication interleaved sequence parallelism.
  - 8 cores = 2 batches x 4 cores/batch.
  - Core j of a batch owns query rows in g=64-row blocks strided by 4:
    global blocks {j, j+4, ..., j+28} (512 rows).  SPMD: one program,
    per-core data; causal structure is identical across cores.
  - Each core computes K/V for its full batch (redundant 4x, but cheaper
    than any on-chip collective here).
  - Scores are computed transposed (S^T: keys on partitions, queries on
    the free axis) so softmax needs no transposes; no max-subtraction
    (scores are O(+-10); exp is fp32-safe).

Perf notes (cost-model driven):
  - fp32 and fp32r share a bit layout: all fp32 DRAM data is DMA'd once
    and bitcast to fp32r at matmul time (no convert copies).
  - wk/wv are pre-rounded to bf16 on the host and shipped as uint16 bit
    patterns (bf16 moving operands run at full PE rate at any tile size).
  - The causal mask is added on the PE itself (identity-stationary matmul
    with a bf16 mask as the moving operand) inside the score accumulation
    group, keeping DVE off the attention critical path.
  - Attention is software-pipelined: score+mask+exp for block i issue
    ahead of the pl/pctx consumption of block i-2, so the in-order PE
    queue never waits on the ACT exp.
  - LayerNorm stats use DVE bn_stats/bn_aggr (one pass, no ACT square).
  - All constants (ones, identities, eps) come from one host tensor: the
    Pool engine issues only SWDGE DMAs, and no engine idles on memsets.
"""

import sys

sys.path.insert(0, "/opt/trn_rl_repo")

import numpy as np

from contextlib import ExitStack
from dataclasses import dataclass, field

from concourse import bacc, mybir, tile

F32 = mybir.dt.float32
R = mybir.dt.float32r
BF = mybir.dt.bfloat16
U16 = mybir.dt.uint16
NEG = -1.0e9
AF = mybir.ActivationFunctionType


@dataclass(frozen=True)
class Cfg:
    L: int = 2048          # sequence length (per batch)
    D: int = 2048          # model dim
    H: int = 16            # query heads
    KV: int = 4            # kv heads
    E: int = 128           # head dim (= partition width)
    mm_dt: object = field(default=mybir.dt.float32r)
    act: object = field(default=None)  # None -> exact GELU
    trivial_affine: bool = False  # gamma==1, beta==0, bo==0: skip those ops

    @property
    def g(self):           # q block granularity (32 blocks across L)
        return self.L // 32

    @property
    def KB(self):          # key block size = 4*g
        return self.L // 8

    @property
    def KSS(self):         # key subtile (partition) size
        return min(self.KB, 128)

    @property
    def ST(self):          # key subtiles per key block
        return max(1, self.KB // 128)

    @property
    def QR(self):          # query rows per core
        return self.L // 4

    @property
    def KT(self):          # contraction tiles over D
        return self.D // 128

    @property
    def RT(self):          # 128-row tiles of the core's q rows
        return self.QR // 128

    @property
    def OC(self):          # out-proj / LN column chunk
        return min(self.D, 512)


def build_program(cfg: Cfg):
    """Build the single-core SPMD Bass program. Returns finalized nc."""
    L, D, H, KV, E = cfg.L, cfg.D, cfg.H, cfg.KV, cfg.E
    g, KB, KSS, ST, QR, KT, RT = (cfg.g, cfg.KB, cfg.KSS, cfg.ST, cfg.QR,
                                  cfg.KT, cfg.RT)
    OC = cfg.OC
    NOC = D // OC
    KVE = KV * E
    act_fn = cfg.act if cfg.act is not None else AF.Gelu
    inv_sqrt_e = 1.0 / float(np.sqrt(E))

    nc = bacc.Bacc(None, target_bir_lowering=False)

    # ---- DRAM I/O (per-core data; same names on every core) ----
    xtu = nc.dram_tensor("xtu", [D, L], U16, kind="ExternalInput")    # x[b].T bf16
    xtqu = nc.dram_tensor("xtqu", [D, QR], U16, kind="ExternalInput")  # bf16
    xq = nc.dram_tensor("xq", [QR, D], F32, kind="ExternalInput")     # rows at q rows
    wqu = nc.dram_tensor("wqu", [D, H * E], U16, kind="ExternalInput")  # bf16
    wku = nc.dram_tensor("wku", [D, KVE], U16, kind="ExternalInput")  # bf16 bits
    wvu = nc.dram_tensor("wvu", [D, KVE], U16, kind="ExternalInput")  # bf16 bits
    wou = nc.dram_tensor("wou", [H * E, D], U16, kind="ExternalInput")  # bf16
    bo2 = nc.dram_tensor("bo2", [2, D], F32, kind="ExternalInput")  # bo row + zero row
    gmb = nc.dram_tensor("gmb", [128, D], F32, kind="ExternalInput")  # gamma bcast
    btb = nc.dram_tensor("btb", [128, D], F32, kind="ExternalInput")  # beta bcast
    # combined f32 consts: [0:130] ones, [130:258] unused, [258] eps,
    # [384:400] bqT, [400:404] bkT, [404:916] bvb  (one DMA)
    cstA = nc.dram_tensor("cstA", [128, 916], F32, kind="ExternalInput")
    # combined bf16-bit consts: [0:128] identity, [128:256] maskd (S^T,
    # st-major), [256:2304] maskp ((kb,st)-major)  (one DMA)
    cstB = nc.dram_tensor("cstB", [128, 2304], U16, kind="ExternalInput")
    out = nc.dram_tensor("out", [QR, D], F32, kind="ExternalOutput")

    with tile.TileContext(nc) as tc, ExitStack() as top:
        # ---- persistent pools (stack order matters for SBUF reuse) ----
        const = top.enter_context(tc.tile_pool(name="const", bufs=1))
        qt_stack = top.enter_context(ExitStack())
        qt_pool = qt_stack.enter_context(tc.tile_pool(name="qtp", bufs=1))
        kvq_pool = top.enter_context(tc.tile_pool(name="kvq", bufs=1))
        xtq_stack = ExitStack()
        xtq_pool = xtq_stack.enter_context(tc.tile_pool(name="xtqp", bufs=1))
        wq_stack = ExitStack()
        wq_pool = wq_stack.enter_context(
            tc.tile_pool(name="wqstage", bufs=3))

        # constants (two DMAs from host; no memsets anywhere)
        cstf_t = const.tile([128, 916], F32)
        cstb_t = const.tile([128, 2304], U16)
        warm = const.tile([1, 2], F32)

        ones_r = const.tile([128, 130], R)

        def load_consts():
            # issued on the sync queue after the first weight/x chunks so
            # the DMA pipe serves the first matmuls' data first
            nc.sync.dma_start(out=cstf_t[:], in_=cstA[:])
            nc.sync.dma_start(out=cstb_t[:], in_=cstB[:])
            # fp32r matmul operands must be produced by a rounding
            # instruction (BIR verifier rule): one tiny convert.
            nc.vector.tensor_copy(ones_r[:], cstf_t[:, 0:130])
            # Prime the Exp activation-table set before any other ACT op so
            # one loaded set covers Copy/Identity/Exp through phase 3.
            nc.scalar.activation(warm[:], cstf_t[:1, 0:2], AF.Exp)
        bq_t = cstf_t[:, 384:400]
        identb = cstb_t[:, 0:128]
        ones2 = ones_r[:, 0:2]          # [128, 2] ones (pl lhsT)
        ones1 = ones_r[:1, 2:130]       # [1, 128] ones (broadcast lhsT)
        eps_c = cstf_t[:, 258:259]      # [128, 1] eps

        # persistent activations: K^T, V (natural) per kv head; Q^T per head
        kT = [kvq_pool.tile([E, L], R, tag=f"kT{kv}", name=f"kT{kv}")
              for kv in range(KV)]
        vN = [kvq_pool.tile([KSS, L // KSS, E], R, tag=f"vN{kv}",
                            name=f"vN{kv}") for kv in range(KV)]

        # x^T at q rows, prefetched during phase 1 (bf16 bits)
        xtq_t = xtq_pool.tile([128, KT, QR], U16)

        # ================= Phase 1: K/V projections (full batch rows) ======
        with ExitStack() as ph:
            wkv_pool = ph.enter_context(tc.tile_pool(name="wkv", bufs=1))
            stage = ph.enter_context(tc.tile_pool(name="stage1", bufs=3))
            cst1 = ph.enter_context(tc.tile_pool(name="cst1", bufs=1))
            ps1 = ph.enter_context(tc.tile_pool(name="ps1", bufs=2, space="PSUM"))

            wk_t = wkv_pool.tile([128, KT, KVE], U16, name="wk_t")
            wv_t = wkv_pool.tile([128, KT, KVE], U16, name="wv_t")

            def load_wkv(c):
                nc.sync.dma_start(
                    out=wk_t[:, 4 * c:4 * (c + 1), :],
                    in_=wku[c * 512:(c + 1) * 512, :]
                    .rearrange("(k p) c -> p k c", p=128))
                nc.sync.dma_start(
                    out=wv_t[:, 4 * c:4 * (c + 1), :],
                    in_=wvu[c * 512:(c + 1) * 512, :]
                    .rearrange("(k p) c -> p k c", p=128))

            load_wkv(0)
            bkT_t = cstf_t[:, 400:404]
            bvb_t = cstf_t[:, 404:916]

            xs2 = None
            for rt in range(L // 128):
                if rt % 2 == 0:
                    xs2 = stage.tile([128, KT, 256], U16, tag="xs")
                    nc.sync.dma_start(
                        out=xs2[:],
                        in_=xtu[:, rt * 128:(rt + 2) * 128]
                        .rearrange("(k p) r -> p k r", p=128))
                xs = xs2[:, :, (rt % 2) * 128:(rt % 2 + 1) * 128]
                # Remaining weight chunks are emitted right after xs(0) (so
                # every consumer follows its producer in program order), but
                # land in the DMA pipe after it; xtq prefetch follows later.
                if rt == 0:
                    load_consts()
                    for c in (1, 2, 3):
                        load_wkv(c)
                elif 4 <= rt <= 7:
                    c = rt - 4
                    nc.sync.dma_start(
                        out=xtq_t[:, 4 * c:4 * (c + 1), :],
                        in_=xtqu[c * 512:(c + 1) * 512, :]
                        .rearrange("(k p) r -> p k r", p=128))
                # K^T computed directly (wk stationary, x^T moving): no
                # PE transposes, and the eviction carries the bias.  Each kv
                # head accumulates in its own PSUM bank: a matmul's start
                # flag pending-zeroes the whole 2KB bank, so independent
                # accumulations must not share one.
                pV = ps1.tile([128, KVE], F32, tag="pV")
                pKTs = [ps1.tile([E, 128], F32, tag=f"pKT{kv}", bufs=1,
                                 name=f"pKT{kv}") for kv in range(KV)]
                for kt in range(KT):
                    nc.tensor.matmul(pV[:], xs[:, kt, :].bitcast(BF),
                                     wv_t[:, kt, :].bitcast(BF),
                                     start=(kt == 0), stop=(kt == KT - 1))
                for kt in range(KT):
                    for kv in range(KV):
                        nc.tensor.matmul(
                            pKTs[kv][:],
                            wk_t[:, kt, kv * E:(kv + 1) * E].bitcast(BF),
                            xs[:, kt, :].bitcast(BF),
                            start=(kt == 0), stop=(kt == KT - 1),
                            skip_group_check=True)
                # V natural: evict (+bias) straight into vN, rounding to fp32r
                for kv in range(KV):
                    nc.vector.tensor_add(
                        vN[kv][:, rt, :], pV[:, kv * E:(kv + 1) * E],
                        bvb_t[:, kv * E:(kv + 1) * E])
                for kv in range(KV):
                    nc.scalar.activation(
                        kT[kv][:, rt * 128:(rt + 1) * 128], pKTs[kv][:],
                        AF.Identity, bias=bkT_t[:, kv:kv + 1])

        # ================= Phase 2: Q^T projection (core's rows) ===========
        qT = [qt_pool.tile([E, QR], R, tag=f"qT{h}", name=f"qT{h}")
              for h in range(H)]
        with ExitStack() as ph:
            ps2 = ph.enter_context(tc.tile_pool(name="ps2", bufs=1, space="PSUM"))
            HB = 4
            for hb in range(H // HB):
                pqs = [ps2.tile([E, QR], F32, tag=f"pq{hh}", name=f"pq{hh}")
                       for hh in range(HB)]
                for c in range(KT // 4):
                    wqs = wq_pool.tile([128, 4, HB * E], U16, tag="wqs")
                    nc.sync.dma_start(
                        out=wqs[:],
                        in_=wqu[c * 512:(c + 1) * 512,
                                hb * HB * E:(hb + 1) * HB * E]
                        .rearrange("(k p) c -> p k c", p=128))
                    for k4 in range(4):
                        kt = 4 * c + k4
                        for hh in range(HB):
                            nc.tensor.matmul(
                                pqs[hh][:],
                                wqs[:, k4, hh * E:(hh + 1) * E].bitcast(BF),
                                xtq_t[:, kt, :].bitcast(BF),
                                start=(kt == 0), stop=(kt == KT - 1))
                for hh in range(HB):
                    # split the evictions across ACT and DVE so the last
                    # group's eviction tail is short
                    h = hb * HB + hh
                    if hh % 2:
                        nc.scalar.activation(
                            qT[h][:], pqs[hh][:], AF.Identity,
                            bias=bq_t[:, h:h + 1])
                    else:
                        nc.vector.tensor_scalar_add(
                            qT[h][:], pqs[hh][:], bq_t[:, h:h + 1])
        wq_stack.close()
        xtq_stack.close()
        # wo prefetch pool: reuses the just-released xtq/wq SBUF region, so
        # its (Pool-queue) DMAs start right after phase 2 and run through
        # phase 3.
        wo_stack = top.enter_context(ExitStack())
        wo_pool = wo_stack.enter_context(tc.tile_pool(name="wop", bufs=2))

        # ================= Phase 3: attention ==============================
        # Flat software pipeline over (head, key-block) steps: the score +
        # mask + exp of step s issue ahead of the pl/pctx consumption of
        # step s-2, and each head's normalize runs inside the next head's
        # stream, so the in-order PE queue never waits on ACT.
        ctxT = [None] * H
        with ExitStack() as ph:
            ps_ctx = ph.enter_context(
                tc.tile_pool(name="psctx", bufs=2, space="PSUM"))
            ps_m = ph.enter_context(tc.tile_pool(name="psm", bufs=2, space="PSUM"))
            ps_s = ph.enter_context(tc.tile_pool(name="pss", bufs=2, space="PSUM"))
            exp_pool = ph.enter_context(tc.tile_pool(name="expp", bufs=4))
            lso_pool = ph.enter_context(tc.tile_pool(name="lso", bufs=2))

            q0s = [min(g * kb, QR // 2) for kb in range(8)]
            qcs = [QR - q0 for q0 in q0s]
            LAG = 2
            steps = [(h, kb) for h in range(H) for kb in range(8)]
            state = {}  # h -> (pl, pctx, ess)

            def produce(h, kb):
                kv = h % KV
                q0, qc = q0s[kb], qcs[kb]
                k0 = kb * KB
                pS = ps_s.tile([KSS, ST, QR], F32, tag="pS")
                for st in range(ST):
                    nc.tensor.matmul(
                        pS[:, st, :qc],
                        kT[kv][:, k0 + st * KSS:k0 + (st + 1) * KSS],
                        qT[h][:, q0:], start=True, stop=False,
                        skip_group_check=True)
                # causal mask folded into the accumulation group on PE:
                # pS[:, st, :w] += I^T @ mask  (bf16 moving, full rate)
                for st in range(ST):
                    if kb < 4:
                        w = g
                        m0 = 128 + st * g
                    else:
                        w = g * (kb - 3)
                        m0 = 256 + ((kb - 4) * ST + st) * (QR // 2)
                    nc.tensor.matmul(
                        pS[:, st, :w], identb.bitcast(BF),
                        cstb_t[:, m0:m0 + w].bitcast(BF), start=False,
                        stop=True, skip_group_check=True)
                eS = exp_pool.tile([KSS, ST, QR], R, tag="eS", bufs=4)
                nc.scalar.activation(eS[:, :, :qc], pS[:, :, :qc], AF.Exp,
                                     scale=inv_sqrt_e)
                state[h][2].append(eS)

            def consume(h, j):
                kv = h % KV
                pl, pctx, ess = state[h]
                q0, qc = q0s[j], qcs[j]
                for st in range(ST):
                    first = (j == 0 and st == 0)
                    lst = (j == 7 and st == ST - 1)
                    nc.tensor.matmul(
                        pl[:, q0:], ones2, ess[j][:, st, :qc],
                        start=first, stop=lst, skip_group_check=True)
                    nc.tensor.matmul(
                        pctx[:, q0:], vN[kv][:, 2 * j + st, :],
                        ess[j][:, st, :qc],
                        start=first, stop=lst, skip_group_check=True)

            def epilogue(h):
                # normalize: cT = pctx * broadcast(1/l)
                pl, pctx, _ = state.pop(h)
                rl = lso_pool.tile([2, QR], R, tag="rl")
                rlf = lso_pool.tile([1, QR], F32, tag="rlf")
                nc.vector.reciprocal_approx_fast(rlf[:], pl[:1, :])
                nc.vector.tensor_copy(rl[:1, :], rlf[:])
                prb = ps_m.tile([E, QR], F32, tag="m")
                nc.tensor.matmul(prb[:], ones1, rl[:1, :],
                                 start=True, stop=True)
                rb_s = lso_pool.tile([E, QR], F32, tag="rbs")
                nc.scalar.activation(rb_s[:], prb[:], AF.Copy)
                cT = qt_pool.tile([E, QR], BF, tag=f"qT{h}", name=f"cT{h}")
                nc.vector.tensor_mul(cT[:], pctx[:], rb_s[:])
                ctxT[h] = cT

            for s in range(len(steps) + LAG):
                if s < len(steps):
                    h, kb = steps[s]
                    if kb == 0:
                        state[h] = (
                            ps_m.tile([2, QR], F32, tag="m", name=f"pl{h}"),
                            ps_ctx.tile([E, QR], F32, tag="pctx",
                                        name=f"pctx{h}"),
                            [])
                    produce(h, kb)
                if s >= LAG:
                    h, j = steps[s - LAG]
                    consume(h, j)
                    if j == 7:
                        epilogue(h)

        # ============ Phase 4: out-proj + GELU + residual + LayerNorm ======
        r_stack = top.enter_context(ExitStack())
        rfull_pool = r_stack.enter_context(tc.tile_pool(name="rfull", bufs=1))
        stat4 = r_stack.enter_context(tc.tile_pool(name="stat4", bufs=1))
        r_full = [rfull_pool.tile([128, D], F32, tag=f"rf{rt}", name=f"rf{rt}")
                  for rt in range(RT)]
        stat6 = [stat4.tile([128, NOC, 6], F32, tag=f"st{rt}", name=f"st{rt}")
                 for rt in range(RT)]
        with ExitStack() as ph:
            ps_y = ph.enter_context(tc.tile_pool(name="psy", bufs=2, space="PSUM"))
            ep_pool = ph.enter_context(tc.tile_pool(name="epp", bufs=3))
            cst4 = ph.enter_context(tc.tile_pool(name="cst4", bufs=1))
            ln_pool = ph.enter_context(tc.tile_pool(name="lnp", bufs=2))
            st_pool = ph.enter_context(tc.tile_pool(name="stp", bufs=2))
            gb_pool = ph.enter_context(tc.tile_pool(name="gbp", bufs=2))

            if not cfg.trivial_affine:
                bo2f = cst4.tile([2, D], F32)
                nc.sync.dma_start(out=bo2f[:], in_=bo2[:])
                bo2r = cst4.tile([2, D], R)
                nc.vector.tensor_copy(bo2r[:], bo2f[:])

            # LayerNorm epilogue.  rstd = rsqrt(var+eps) is computed per
            # row-tile on the DVE via Newton iterations seeded from 1/v
            # (3 iters: rel err <3e-5 for v near 1.5; converges v>1/3), so no ACT
            # Sqrt is needed: the Gelu table set stays loaded, and each
            # row-tile normalizes + stores as soon as its own stats land.
            mv4 = st_pool.tile([128, RT, 2], F32, name="mv4")

            def ln_rt(rt):
                nc.vector.bn_aggr(mv4[:, rt, :], stat6[rt][:])
                vv = st_pool.tile([128, 1], F32, tag=f"vv{rt}", name=f"vv{rt}")
                nc.vector.tensor_scalar_add(vv[:], mv4[:, rt, 1:2], eps_c)
                y = st_pool.tile([128, 1], F32, tag=f"y{rt}", name=f"y{rt}")
                nc.vector.reciprocal(y[:], vv[:])
                t = st_pool.tile([128, 1], F32, tag=f"t{rt}", name=f"t{rt}")
                for _ in range(3):
                    nc.vector.tensor_mul(t[:], y[:], y[:])
                    nc.vector.tensor_mul(t[:], t[:], vv[:])
                    nc.vector.tensor_scalar(
                        t[:], t[:], -0.5, 1.5,
                        op0=mybir.AluOpType.mult, op1=mybir.AluOpType.add)
                    nc.vector.tensor_mul(y[:], y[:], t[:])
                nmr = st_pool.tile([128, 1], F32, tag=f"nm{rt}",
                                   name=f"nm{rt}")
                nc.vector.tensor_mul(nmr[:], mv4[:, rt, 0:1], y[:])
                nc.vector.tensor_scalar_mul(nmr[:], nmr[:], -1.0)
                for c in range(NOC):
                    sl = slice(c * OC, (c + 1) * OC)
                    rchunk = r_full[rt][:, sl]
                    if (c + rt) % 2:
                        nc.scalar.activation(
                            rchunk, rchunk, AF.Identity,
                            scale=y[:], bias=nmr[:])
                    else:
                        nc.vector.tensor_scalar(
                            rchunk, rchunk, y[:], nmr[:],
                            op0=mybir.AluOpType.mult, op1=mybir.AluOpType.add)
                    if not cfg.trivial_affine:
                        gm_c = gb_pool.tile([128, OC], F32, tag="gmc")
                        bt_c = gb_pool.tile([128, OC], F32, tag="btc")
                        nc.sync.dma_start(out=gm_c[:], in_=gmb[:, sl])
                        nc.sync.dma_start(out=bt_c[:], in_=btb[:, sl])
                        nc.vector.tensor_mul(rchunk, rchunk, gm_c[:])
                        nc.vector.tensor_add(rchunk, rchunk, bt_c[:])
                    nc.sync.dma_start(out=out[rt * 128:(rt + 1) * 128, sl],
                                      in_=rchunk)

            HW4 = 4  # h-chunk per wo load piece
            for oc in range(NOC):
                # load wo[:, oc] transposed-tiled: (128, H, OC) in pieces
                woc = wo_pool.tile([128, H, OC], U16, tag="woc")
                for pc in range(H // HW4):
                    nc.gpsimd.dma_start(
                        out=woc[:, pc * HW4:(pc + 1) * HW4, :],
                        in_=wou[pc * HW4 * E:(pc + 1) * HW4 * E,
                                oc * OC:(oc + 1) * OC]
                        .rearrange("(h p) c -> p h c", p=128))
                for rt in range(RT):
                    py = ps_y.tile([128, OC], F32, tag="py")
                    for h in range(H):
                        nc.tensor.matmul(
                            py[:], ctxT[h][:, rt * 128:(rt + 1) * 128],
                            woc[:, h, :].bitcast(BF), start=(h == 0),
                            stop=(cfg.trivial_affine and h == H - 1))
                    if not cfg.trivial_affine:
                        nc.tensor.matmul(
                            py[:], ones1,
                            bo2r[:1, oc * OC:(oc + 1) * OC],
                            start=False, stop=True, skip_group_check=True)
                    t2 = ep_pool.tile([128, OC], F32, tag="t2")
                    nc.scalar.activation(t2[:], py[:], act_fn)
                    xqt = ep_pool.tile([128, OC], F32, tag="xqt")
                    nc.sync.dma_start(
                        out=xqt[:],
                        in_=xq[rt * 128:(rt + 1) * 128, oc * OC:(oc + 1) * OC])
                    rchunk = r_full[rt][:, oc * OC:(oc + 1) * OC]
                    nc.vector.tensor_add(rchunk, t2[:], xqt[:])
                    nc.vector.bn_stats(stat6[rt][:, oc, :], rchunk)
                    if oc == NOC - 1:
                        ln_rt(rt)

    nc.finalize()
    return nc


# ---------------------------------------------------------------------------
# host-side mask construction + sharding
# ---------------------------------------------------------------------------

def _bf16_bits(a):
    u = np.ascontiguousarray(a, np.float32).view(np.uint32)
    return ((u + 0x8000) >> 16).astype(np.uint16)


def build_masks(cfg: Cfg, j: int):
    g, KB, QR, KSS, ST = cfg.g, cfg.KB, cfg.QR, cfg.KSS, cfg.ST
    c = np.arange(KB)[:, None]
    r = np.arange(g)[None, :]
    maskd = np.where(c <= j * g + r, 0.0, NEG).astype(np.float32)
    maskp = np.zeros((4, KB, QR // 2), np.float32)
    m = np.arange(QR // 2)
    i_of_m = 4 + m // g
    r_of_m = m % g
    for kbi, kb in enumerate(range(4, 8)):
        block = np.zeros((KB, QR // 2), np.float32)
        block[:, i_of_m < kb] = NEG
        dcols = np.where(i_of_m == kb)[0]
        block[:, dcols] = np.where(c <= j * g + r_of_m[dcols][None, :], 0.0, NEG)
        maskp[kbi] = block
    # rearrange to partitioned S^T layout and convert to bf16 bit patterns
    maskdu = _bf16_bits(maskd.reshape(ST, KSS, g).transpose(1, 0, 2))
    maskpu = _bf16_bits(
        maskp.reshape(4, ST, KSS, QR // 2).transpose(2, 0, 1, 3))
    return (np.ascontiguousarray(maskdu.reshape(KSS, ST * g)),
            np.ascontiguousarray(maskpu.reshape(KSS, 4 * ST * (QR // 2))))


def q_rows(cfg: Cfg, j: int):
    g = cfg.g
    return np.concatenate(
        [np.arange((j + 4 * i) * g, (j + 4 * i + 1) * g) for i in range(8)])


def make_in_map(cfg: Cfg, shared, x, b, j):
    rows = q_rows(cfg, j)
    xb = np.asarray(x, np.float32)[b]
    xbT = np.ascontiguousarray(xb.T)
    maskdu, maskpu = build_masks(cfg, j)
    cstB = np.empty((cfg.KSS, 2304), np.uint16)
    cstB[:, 0:128] = shared["_identu"]
    cstB[:, 128:256] = maskdu
    cstB[:, 256:2304] = maskpu
    d = dict(
        shared,
        xtu=_bf16_bits(xbT),
        xtqu=_bf16_bits(xbT[:, rows]),
        xq=np.ascontiguousarray(xb[rows]),
        cstB=cstB,
    )
    del d["_identu"]
    return d


def make_shared(cfg: Cfg, Wq, bq, Wk, bk, Wv, bv, Wo, bo, gamma, beta):
    H, KV, E, D = cfg.H, cfg.KV, cfg.E, cfg.D
    cstA = np.zeros((128, 916), np.float32)
    cstA[:, :130] = 1.0
    cstA[:, 258] = 1e-5
    cstA[:, 384:400] = np.asarray(bq, np.float32).reshape(H, E).T
    cstA[:, 400:404] = np.asarray(bk, np.float32).reshape(KV, E).T
    cstA[:, 404:916] = np.asarray(bv, np.float32)[None, :]
    return {
        "wqu": _bf16_bits(Wq),
        "wku": _bf16_bits(Wk),
        "wvu": _bf16_bits(Wv),
        "wou": _bf16_bits(Wo),
        "bo2": np.ascontiguousarray(
            np.stack([np.asarray(bo, np.float32),
                      np.zeros(D, np.float32)])),
        "gmb": np.ascontiguousarray(
            np.broadcast_to(np.asarray(gamma, np.float32), (128, D))),
        "btb": np.ascontiguousarray(
            np.broadcast_to(np.asarray(beta, np.float32), (128, D))),
        "cstA": cstA,
        "_identu": _bf16_bits(np.eye(128, dtype=np.float32)),
    }


def assemble(cfg: Cfg, results, B):
    out = np.empty((B, cfg.L, cfg.D), np.float32)
    for core in range(4 * B):
        b, j = divmod(core, 4)
        out[b, q_rows(cfg, j)] = results[core]["out"]
    return out


_NC_CACHE = {}


def kernel(x, Wq, bq, Wk, bk, Wv, bv, Wo, bo, gamma, beta):
    from concourse.bass_utils import run_bass_kernel_spmd

    trivial = bool(
        np.all(np.asarray(gamma) == 1.0) and np.all(np.asarray(beta) == 0.0)
        and np.all(np.asarray(bo) == 0.0))
    cfg = Cfg(trivial_affine=trivial)
    if cfg not in _NC_CACHE:
        _NC_CACHE[cfg] = build_program(cfg)
    nc = _NC_CACHE[cfg]
    shared = make_shared(cfg, Wq, bq, Wk, bk, Wv, bv, Wo, bo, gamma, beta)
    in_maps = [make_in_map(cfg, shared, x, *divmod(core, 4))
               for core in range(8)]
    res = run_bass_kernel_spmd(nc, in_maps, list(range(8)))
    return assemble(cfg, res.results, 2)
